# Initial kernel scaffold
#
"""Trainium2 Bass kernel for nn_BeeGameModule (histogram_binning).

Reference computation (per batch b of 4096):
    agent_vote[b,a] = argmax_h votes[b,a,h]          (A=128 agents, H=64 hives)
    counts[b,h]     = histogram of agent_vote[b,:]
    max_freq[b]     = counts.max() / 128
    value[b]        = sum_a hive_values[b, agent_vote[b,a]]
                    = sum_h counts[b,h] * hive_values[b,h]
    discount[b]     = 100*(1 - sigmoid(30*(max_freq[b] - 0.7)))
    vote_cost       = -sum_b value[b]/discount[b]
    movement_cost   = sum_{b,e} ||movements[b,e,:]||_2
    out             = (vote_cost + movement_cost, max_freq)

Key identity: with no argmax ties (true for these inputs), the one-hot
eq[b,a,h] = (votes[b,a,h] == max_h votes[b,a,:]) gives counts exactly via a
sum over agents, and value via counts . hive_values.

Sharding: pure data parallel over the batch axis across 8 cores; host sums
the 8 per-core scalar partials and concatenates max_freq shards.
"""

import numpy as np

B = 4096
A = 128          # agents
H = 64           # hives
E = 192          # entities
NCORES = 8
BC = B // NCORES         # 512 batches per core
P = 128                  # SBUF partitions
TILES = BC // P          # 4 tiles of 128 batches
D_DISC, K_DISC, T_DISC = 100.0, 30.0, 0.7

_CACHE = {}


def _build_bass():
    import concourse.bass as bass
    import concourse.mybir as mybir
    from concourse.tile import TileContext
    from contextlib import ExitStack

    f32 = mybir.dt.float32
    bf16 = mybir.dt.bfloat16
    X = mybir.AxisListType.X
    Alu = mybir.AluOpType

    nc = bass.Bass()
    votes = nc.declare_dram_parameter("votes", [BC, A * H], f32, isOutput=False)
    mov = nc.declare_dram_parameter("movements", [BC, E * 2], f32, isOutput=False)
    hv = nc.declare_dram_parameter("hive_values", [BC, H], f32, isOutput=False)
    out_mf = nc.declare_dram_parameter("max_freq", [BC], f32, isOutput=True)
    out_part = nc.declare_dram_parameter("partial", [P], f32, isOutput=True)

    with ExitStack() as ctx, TileContext(nc) as tc:
        main = ctx.enter_context(tc.tile_pool(name="main", bufs=2))
        small = ctx.enter_context(tc.tile_pool(name="small", bufs=2))
        accp = ctx.enter_context(tc.tile_pool(name="accp", bufs=1))

        # acc columns: [0:TILES] = value/discount per tile, [TILES:2*TILES] = movement sums
        acc = accp.tile([P, 2 * TILES], f32)

        for t in range(TILES):
            ts = bass.ts(t, P)

            # ---- votes tile: [128 batches, 128 agents * 64 hives] ----
            v = main.tile([P, A * H], f32, tag="v")
            nc.sync.dma_start(out=v, in_=votes[ts])
            v3 = v.rearrange("p (a h) -> p a h", h=H)

            # per-agent max over hives
            m = small.tile([P, A], f32, tag="m")
            nc.vector.reduce_max(m, v3, axis=X)

            # one-hot: eq[b,a,h] = (votes == m) in bf16 (exact 0.0/1.0)
            mb = m.rearrange("p (a o) -> p a o", o=1).broadcast_to([P, A, H])
            eq = main.tile([P, A * H], bf16, tag="eq")
            nc.vector.tensor_tensor(
                eq.rearrange("p (a h) -> p a h", h=H), v3, mb, Alu.is_equal
            )

            # tree-sum over agents: halve [P, n] until counts [P, H]
            cur = eq
            n = A * H // 2
            while n >= H:
                nxt = main.tile([P, n], f32 if n == H else bf16, tag=f"tree{n}",
                                name=f"tree{n}_{t}")
                nc.vector.tensor_add(nxt, cur[:, :n], cur[:, n:2 * n])
                cur = nxt
                n //= 2
            counts = cur  # [P, H] f32, exact integers

            # max_freq = counts.max()/128
            mf_raw = small.tile([P, 1], f32, tag="mf_raw")
            nc.vector.reduce_max(mf_raw, counts.rearrange("p (o h) -> p o h", o=1),
                                 axis=X)
            mf_out = small.tile([P, 1], f32, tag="mf_out")
            nc.scalar.mul(mf_out, mf_raw, 1.0 / A)
            nc.sync.dma_start(out=out_mf[ts], in_=mf_out)

            # value = sum_h counts*hv  (fused mult + accumulate)
            hvt = small.tile([P, H], f32, tag="hvt")
            nc.sync.dma_start(out=hvt, in_=hv[ts])
            prod = small.tile([P, H], f32, tag="prod")
            value = small.tile([P, 1], f32, tag="value")
            nc.vector.scalar_tensor_tensor(prod, counts, 0.0, hvt,
                                           Alu.add, Alu.mult, accum_out=value)

            # discount = 100*(1 - sigmoid(30*(mf_raw/128 - 0.7)))
            sg = small.tile([P, 1], f32, tag="sg")
            nc.scalar.activation(sg, mf_raw, mybir.ActivationFunctionType.Sigmoid,
                                 bias=-K_DISC * T_DISC, scale=K_DISC / A)
            denom = small.tile([P, 1], f32, tag="denom")
            nc.scalar.activation(denom, sg, mybir.ActivationFunctionType.Identity,
                                 bias=D_DISC, scale=-D_DISC)
            recip = small.tile([P, 1], f32, tag="recip")
            nc.vector.reciprocal(recip, denom)
            nc.vector.tensor_tensor(acc[:, t:t + 1], value, recip, Alu.mult)

            # ---- movement: sum of L2 norms ----
            mv = main.tile([P, E * 2], f32, tag="mv")
            nc.sync.dma_start(out=mv, in_=mov[ts])
            sq = main.tile([P, E * 2], f32, tag="sq")
            nc.scalar.square(sq, mv)
            sq3 = sq.rearrange("p (e two) -> p e two", two=2)
            ps = main.tile([P, E], f32, tag="ps")
            nc.vector.tensor_tensor(ps, sq3[:, :, 0], sq3[:, :, 1], Alu.add)
            rt = main.tile([P, E], f32, tag="rt")
            nc.scalar.activation(rt, ps, mybir.ActivationFunctionType.Sqrt,
                                 accum_out=acc[:, TILES + t:TILES + t + 1])

        # per-partition total: sum(movement) - sum(value/discount)
        vsum = accp.tile([P, 1], f32)
        nc.vector.reduce_sum(vsum, acc[:, 0:TILES].rearrange("p (o t) -> p o t", o=1),
                             axis=X)
        msum = accp.tile([P, 1], f32)
        nc.vector.reduce_sum(msum, acc[:, TILES:2 * TILES]
                             .rearrange("p (o t) -> p o t", o=1), axis=X)
        tot = accp.tile([P, 1], f32)
        nc.vector.tensor_tensor(tot, msum, vsum, Alu.subtract)
        nc.sync.dma_start(out=out_part[:], in_=tot)

    return nc


def kernel(movements, utterances, votes, hive_values, locations):
    from concourse.bass_utils import run_bass_kernel_spmd

    if "nc" not in _CACHE:
        _CACHE["nc"] = _build_bass()
    nc = _CACHE["nc"]

    votes = np.ascontiguousarray(votes, dtype=np.float32)
    movements = np.ascontiguousarray(movements, dtype=np.float32)
    hive_values = np.ascontiguousarray(hive_values, dtype=np.float32)

    in_maps = []
    for c in range(NCORES):
        sl = slice(c * BC, (c + 1) * BC)
        in_maps.append({
            "votes": votes[sl].reshape(BC, A * H),
            "movements": movements[sl].reshape(BC, E * 2),
            "hive_values": hive_values[sl].reshape(BC, H),
        })

    res = run_bass_kernel_spmd(nc, in_maps, core_ids=list(range(NCORES)))
    _CACHE["last_result"] = res

    max_freq = np.concatenate([r["max_freq"] for r in res.results])
    total = np.float32(np.sum(np.float64(
        np.concatenate([r["partial"] for r in res.results]))))
    return (total, max_freq)


# revision 10
# speedup vs baseline: 1.2078x; 1.2078x over previous
"""Trainium2 Bass kernel for nn_BeeGameModule (histogram_binning).

Reference computation (per batch b of 4096):
    agent_vote[b,a] = argmax_h votes[b,a,h]          (A=128 agents, H=64 hives)
    counts[b,h]     = histogram of agent_vote[b,:]
    max_freq[b]     = counts.max() / 128
    value[b]        = sum_a hive_values[b, agent_vote[b,a]]
                    = sum_h counts[b,h] * hive_values[b,h]
    discount[b]     = 100*(1 - sigmoid(30*(max_freq[b] - 0.7)))
    vote_cost       = -sum_b value[b]/discount[b]
    movement_cost   = sum_{b,e} ||movements[b,e,:]||_2
    out             = (vote_cost + movement_cost, max_freq)

Key identity: with no argmax ties (true for these inputs), the one-hot
eq[b,a,h] = (votes[b,a,h] == max_h votes[b,a,:]) gives counts exactly via a
sum over agents, and value via counts . hive_values.

Sharding: pure data parallel over the batch axis across 8 cores; host sums
the 8 per-core scalar partials and concatenates max_freq shards.
"""

import numpy as np

B = 4096
A = 128          # agents
H = 64           # hives
E = 192          # entities
NCORES = 8
BC = B // NCORES         # 512 batches per core
P = 128                  # SBUF partitions
TILES = BC // P          # 4 tiles of 128 batches
D_DISC, K_DISC, T_DISC = 100.0, 30.0, 0.7

_CACHE = {}


def _build_bass(repeat: int = 1):
    import concourse.bass as bass
    import concourse.bacc as bacc
    import concourse.mybir as mybir
    from concourse.tile import TileContext
    from contextlib import ExitStack

    f32 = mybir.dt.float32
    bf16 = mybir.dt.bfloat16
    X = mybir.AxisListType.X
    Alu = mybir.AluOpType

    nc = bacc.Bacc()
    votes = nc.declare_dram_parameter("votes", [BC, A * H], f32, isOutput=False)
    mov = nc.declare_dram_parameter("movements", [BC, E * 2], f32, isOutput=False)
    hv = nc.declare_dram_parameter("hive_values", [BC, H], f32, isOutput=False)
    out_mf = nc.declare_dram_parameter("max_freq", [BC], f32, isOutput=True)
    out_part = nc.declare_dram_parameter("partial", [P], f32, isOutput=True)

    with TileContext(nc) as tc, ExitStack() as ctx:
        main = ctx.enter_context(tc.tile_pool(name="main", bufs=2))
        small = ctx.enter_context(tc.tile_pool(name="small", bufs=2))
        accp = ctx.enter_context(tc.tile_pool(name="accp", bufs=1))

        # acc columns: [0:TILES] = value/discount per tile, [TILES:2*TILES] = movement sums
        acc = accp.tile([P, 2 * TILES], f32)
        mf_all = accp.tile([P, TILES], f32)

        for rep in range(repeat):
          for t in range(TILES):
            ts = bass.ts(t, P)

            # ---- votes tile: [128 batches, 128 agents * 64 hives] ----
            v = main.tile([P, A * H], f32, tag="v", bufs=TILES)
            nc.sync.dma_start(out=v, in_=votes[ts])
            v3 = v.rearrange("p (a h) -> p a h", h=H)

            # per-agent max over hives
            m = small.tile([P, A], f32, tag="m")
            nc.vector.reduce_max(m, v3, axis=X)

            # one-hot: eq[b,a,h] = (votes == m) in bf16 (exact 0.0/1.0)
            mb = m.rearrange("p (a o) -> p a o", o=1).broadcast_to([P, A, H])
            eq = main.tile([P, A * H], bf16, tag="eq", bufs=1)
            nc.vector.tensor_tensor(
                eq.rearrange("p (a h) -> p a h", h=H), v3, mb, Alu.is_equal
            )

            # tree-sum over agents: halve [P, n] until counts [P, H]
            cur = eq
            n = A * H // 2
            while n >= H:
                nxt = main.tile([P, n], f32 if n == H else bf16, tag=f"tree{n}",
                                name=f"tree{n}_{t}")
                nc.vector.tensor_add(nxt, cur[:, :n], cur[:, n:2 * n])
                cur = nxt
                n //= 2
            counts = cur  # [P, H] f32, exact integers

            # max_freq = counts.max()/128
            mf_raw = small.tile([P, 1], f32, tag="mf_raw")
            nc.vector.reduce_max(mf_raw, counts.rearrange("p (o h) -> p o h", o=1),
                                 axis=X)
            nc.scalar.mul(mf_all[:, t:t + 1], mf_raw, 1.0 / A)

            # value = sum_h counts*hv  (fused mult + accumulate)
            hvt = small.tile([P, H], f32, tag="hvt", bufs=TILES)
            nc.gpsimd.dma_start(out=hvt, in_=hv[ts])
            prod = small.tile([P, H], f32, tag="prod")
            value = small.tile([P, 1], f32, tag="value")
            nc.vector.scalar_tensor_tensor(prod, counts, 0.0, hvt,
                                           Alu.add, Alu.mult, accum_out=value)

            # discount = 100*(1 - sigmoid(30*(mf_raw/128 - 0.7)))
            sgarg = small.tile([P, 1], f32, tag="sgarg")
            nc.vector.tensor_scalar(sgarg, mf_raw, K_DISC / A, -K_DISC * T_DISC,
                                    Alu.mult, Alu.add)
            sg = small.tile([P, 1], f32, tag="sg")
            nc.scalar.activation(sg, sgarg, mybir.ActivationFunctionType.Sigmoid)
            denom = small.tile([P, 1], f32, tag="denom")
            nc.vector.tensor_scalar(denom, sg, -D_DISC, D_DISC, Alu.mult, Alu.add)
            recip = small.tile([P, 1], f32, tag="recip")
            nc.vector.reciprocal(recip, denom)
            nc.vector.tensor_tensor(acc[:, t:t + 1], value, recip, Alu.mult)

            # ---- movement: sum of L2 norms ----
            mv = main.tile([P, E * 2], f32, tag="mv", bufs=TILES)
            nc.gpsimd.dma_start(out=mv, in_=mov[ts])
            sq = main.tile([P, E * 2], f32, tag="sq", bufs=TILES)
            nc.scalar.square(sq, mv)
            sq3 = sq.rearrange("p (e two) -> p e two", two=2)
            ps = main.tile([P, E], f32, tag="ps", bufs=TILES)
            nc.vector.tensor_tensor(ps, sq3[:, :, 0], sq3[:, :, 1], Alu.add)
            rt = main.tile([P, E], f32, tag="rt", bufs=TILES)
            nc.scalar.activation(rt, ps, mybir.ActivationFunctionType.Sqrt,
                                 accum_out=acc[:, TILES + t:TILES + t + 1])

        # per-partition total: sum(movement) - sum(value/discount)
        vsum = accp.tile([P, 1], f32)
        nc.vector.reduce_sum(vsum, acc[:, 0:TILES].rearrange("p (o t) -> p o t", o=1),
                             axis=X)
        msum = accp.tile([P, 1], f32)
        nc.vector.reduce_sum(msum, acc[:, TILES:2 * TILES]
                             .rearrange("p (o t) -> p o t", o=1), axis=X)
        tot = accp.tile([P, 1], f32)
        nc.vector.tensor_tensor(tot, msum, vsum, Alu.subtract)
        nc.gpsimd.dma_start(out=out_part[:], in_=tot)
        nc.sync.dma_start(out=out_mf.rearrange("(t p) -> p t", p=P), in_=mf_all)

    nc.finalize()
    return nc


def kernel(movements, utterances, votes, hive_values, locations):
    from concourse.bass_utils import run_bass_kernel_spmd

    if "nc" not in _CACHE:
        _CACHE["nc"] = _build_bass()
    nc = _CACHE["nc"]

    votes = np.ascontiguousarray(votes, dtype=np.float32)
    movements = np.ascontiguousarray(movements, dtype=np.float32)
    hive_values = np.ascontiguousarray(hive_values, dtype=np.float32)

    in_maps = []
    for c in range(NCORES):
        sl = slice(c * BC, (c + 1) * BC)
        in_maps.append({
            "votes": votes[sl].reshape(BC, A * H),
            "movements": movements[sl].reshape(BC, E * 2),
            "hive_values": hive_values[sl].reshape(BC, H),
        })

    res = run_bass_kernel_spmd(nc, in_maps, core_ids=list(range(NCORES)))
    _CACHE["last_result"] = res

    max_freq = np.concatenate([r["max_freq"] for r in res.results])
    total = np.float32(np.sum(np.float64(
        np.concatenate([r["partial"] for r in res.results]))))
    return (total, max_freq)
